# revision 1
# baseline (speedup 1.0000x reference)
"""MultiHuberLoss Trainium2 kernel.

Reference (per element, with m = +x at the target class, -x elsewhere):
    hinge = max(0, 1 - m);  loss = where(m >= -1, hinge^2, -4m);  out = sum(loss)/N

Math used here (all exact identities):
  F(m) = relu(1-m)^2 - relu(-1-m)^2          (the piecewise huber-hinge)
  Main pass uses m = -x for EVERY element:
      F(-x) = min(relu(x+1)^2, 4) + 4*max(x, 1) - 4
            = (clamp(x, -1, 1) + 1)^2 + 4*max(x, 1) - 4
  Per-row correction for the target column t (where m = +x_t):
      F(x_t) - F(-x_t) = -4 * x_t             (telescopes exactly)

So:  sum(loss) = sum_ij (clamp(x,-1,1)+1)^2 + 4*sum_ij max(x,1) - 4*count
                 - 4 * sum_i x[i, target_i]

Data parallel over 8 cores (8192 rows each). Per core:
  - DVE:  v = clamp(x,-1,1)  (tensor_scalar max,min @ 2x fp32 mode)
  - ACT:  Square(v + 1) with fused accum_out  -> per-partition sums A
  - B-term sum(max(x,1)) split between DVE CACHE_REDUCE tiles and ACT
    relu(x-1)+accum tiles to balance the two engines
  - target extraction split between GPSIMD indirect-DMA gathers (one
    offset per partition per op -- multi-offset indirect DMA is broken
    in this walrus) and DVE scalar_tensor_tensor is_equal masks on
    row-aligned sub-slices, sized so DVE / ACT / gather-chain / DMA all
    finish together.
"""

import numpy as np

import concourse.bacc as bacc
import concourse.bass as bass
import concourse.mybir as mybir
from concourse.bass_utils import run_bass_kernel_spmd
from concourse.tile import TileContext

N_TOTAL = 65536
C = 1000
N_CORES = 8
ROWS = N_TOTAL // N_CORES  # 8192 rows per core
P = 128                    # partitions
JPP = ROWS // P            # 64 rows per partition
FREE = JPP * C             # 64000 f32 per partition
FD = 4000                  # free-dim tile size (4 whole rows per partition)
NT = FREE // FD            # 16 tiles
RPT = FD // C              # rows per partition per tile (4)

# tiles whose sub-rows get their target extracted via DVE is_equal-mask
# (scalar_tensor_tensor) instead of a gpsimd indirect-DMA gather
STT_TILES = (2, 5, 8, 12)
# tiles whose B-term (sum max(x,1)) runs on ACT as relu(x-1)+accum
# (the rest use DVE tensor_scalar max + CACHE_REDUCE); the last tile must
# stay on DVE so the gather-reduce WAW pin lands after the whole V stream
ACT_B_TILES = {0, 2, 4, 5, 7, 8, 10, 12}

f32 = mybir.dt.float32
i32 = mybir.dt.int32
Alu = mybir.AluOpType


def build_program():
    nc = bacc.Bacc(
        "TRN2", target_bir_lowering=False, debug=False, num_devices=N_CORES
    )
    x = nc.dram_tensor("x", [ROWS, C], f32, kind="ExternalInput")
    # host-precomputed flat element offsets: og[r] = r*C + target[r]
    og = nc.dram_tensor("og", [ROWS], i32, kind="ExternalInput")
    # target column of each row as f32 (for the is_equal extraction)
    tc_in = nc.dram_tensor("tc", [ROWS], f32, kind="ExternalInput")
    out = nc.dram_tensor("out", [1, 1], f32, kind="ExternalOutput")

    x_flat = x.ap().rearrange("(p j) c -> p (j c)", p=P)  # [128, 64000]
    x_lin = x.ap().rearrange("a (b one) -> (a b) one", one=1)  # [8192000, 1]
    og2d = og.ap().rearrange("(p j) -> p j", p=P)         # [128, 64]
    tc2d = tc_in.ap().rearrange("(p j) -> p j", p=P)      # [128, 64]

    stt_js = {t * RPT + j for t in STT_TILES for j in range(RPT)}
    n_stt = len(stt_js)

    with TileContext(nc) as tc:
        with (
            tc.tile_pool(name="xp", bufs=4) as xp,
            tc.tile_pool(name="vp", bufs=3) as vp,
            tc.tile_pool(name="scr", bufs=1) as scr,
            tc.tile_pool(name="small", bufs=1) as small,
            tc.tile_pool(name="psp", bufs=1, space="PSUM") as psp,
        ):
            ones = small.tile([P, 1], f32, tag="ones")
            nc.vector.memset(ones[:], 1.0)
            negones = small.tile([P, 1], f32, tag="negones")
            nc.vector.memset(negones[:], -1.0)
            # column-index ramp 0..999, same on every partition (f32 exact)
            ci = small.tile([P, C], f32, tag="ci")
            nc.gpsimd.iota(
                ci[:], pattern=[[1, C]], base=0, channel_multiplier=0,
                allow_small_or_imprecise_dtypes=True,
            )

            # ---- gather path, traced FIRST so the offsets DMA leads the
            # Sync queue and the gathers start on gpsimd at t~8us ----
            offs = small.tile([P, JPP], i32, tag="offs")
            nc.sync.dma_start(out=offs[:], in_=og2d)
            tcv = small.tile([P, JPP], f32, tag="tcv")
            nc.sync.dma_start(out=tcv[:], in_=tc2d)
            G = small.tile([P, JPP], f32, tag="G")
            nc.vector.memset(G[:], 0.0)
            # single-offset-per-partition gathers, ~1.1us Q7 emission each,
            # overlapped under the main-loop DMA stream
            for j in range(JPP):
                if j in stt_js:
                    continue
                nc.gpsimd.indirect_dma_start(
                    out=G[:, j:j + 1],
                    out_offset=None,
                    in_=x_lin,
                    in_offset=bass.IndirectOffsetOnAxis(
                        ap=offs[:, j:j + 1], axis=0
                    ),
                )

            # ---- main streaming loop ----
            # Scratch tags are per-engine ("act_scr" written only by Scalar,
            # "w_dve"/"w_stt" only by Vector) so discarded outputs never
            # create cross-engine WAW serialization.
            accA = small.tile([P, NT], f32, tag="accA")
            accB = small.tile([P, NT], f32, tag="accB")
            gstt = small.tile([P, max(1, n_stt)], f32, tag="gstt")
            si = 0
            for t in range(NT):
                xt = xp.tile([P, FD], f32)
                # two half-tile DMAs: finer SDMA descriptors (8KB) so the
                # concurrent gather descriptors aren't stuck behind big
                # blocking quanta on the shared SDMA engines
                h = FD // 2
                nc.sync.dma_start(
                    out=xt[:, 0:h], in_=x_flat[:, t * FD:t * FD + h]
                )
                nc.sync.dma_start(
                    out=xt[:, h:FD], in_=x_flat[:, t * FD + h:(t + 1) * FD]
                )
                v = vp.tile([P, FD], f32)
                nc.vector.tensor_scalar(
                    v[:], xt[:], -1.0, 1.0, Alu.max, Alu.min
                )
                sq = scr.tile([P, FD], f32, tag="act_scr")
                nc.scalar.activation(
                    sq[:],
                    v[:],
                    mybir.ActivationFunctionType.Square,
                    bias=1.0,
                    scale=1.0,
                    accum_out=accA[:, t:t + 1],
                )
                if t in ACT_B_TILES:
                    # accB col = sum relu(x-1)  ( = sum max(x,1) - FD )
                    w = scr.tile([P, FD], f32, tag="act_scr")
                    nc.scalar.activation(
                        w[:],
                        xt[:],
                        mybir.ActivationFunctionType.Relu,
                        bias=negones[:],
                        scale=1.0,
                        accum_out=accB[:, t:t + 1],
                    )
                else:
                    # accB col = sum max(x,1)
                    w = scr.tile([P, FD], f32, tag="w_dve")
                    nc.vector.tensor_scalar(
                        w[:], xt[:], 1.0, None, Alu.max, Alu.add,
                        accum_out=accB[:, t:t + 1],
                    )
                if t in STT_TILES:
                    # per-sub-row target extraction:
                    #   gstt col = sum( (ci == target_col) * x_subrow )
                    for j in range(RPT):
                        ws = scr.tile([P, C], f32, tag="w_stt")
                        nc.vector.scalar_tensor_tensor(
                            out=ws[:],
                            in0=ci[:],
                            scalar=tcv[:, t * RPT + j:t * RPT + j + 1],
                            in1=xt[:, j * C:(j + 1) * C],
                            op0=Alu.is_equal,
                            op1=Alu.mult,
                            accum_out=gstt[:, si:si + 1],
                        )
                        si += 1

            # ---- final combine ----
            # partial = [ sum_p(rA + 4*(rB - rG)) + 4*|ACT_B|*FD*P
            #             - 4*ROWS*C ] / N_TOTAL
            # rG is written into the "w_dve"-tagged scratch slot: the WAW dep
            # on the last main-loop CR write pins it to the END of the Vector
            # stream (the scheduler's cost model underestimates the gather
            # chain and would otherwise hoist it, head-of-line blocking the
            # main loop -- measured 65us stall).
            rg_w = scr.tile([P, FD], f32, tag="w_dve")
            nc.vector.reduce_sum(rg_w[:, 0:1], G[:], axis=mybir.AxisListType.X)
            nc.vector.reduce_sum(
                rg_w[:, 1:2], gstt[:], axis=mybir.AxisListType.X
            )
            rGt = small.tile([P, 1], f32, tag="rGt")
            nc.vector.tensor_tensor(
                rGt[:], rg_w[:, 0:1], rg_w[:, 1:2], Alu.add
            )
            rA = small.tile([P, 1], f32, tag="rA")
            nc.vector.reduce_sum(rA[:], accA[:], axis=mybir.AxisListType.X)
            rB = small.tile([P, 1], f32, tag="rB")
            nc.vector.reduce_sum(rB[:], accB[:], axis=mybir.AxisListType.X)
            u2 = small.tile([P, 1], f32, tag="u2")
            nc.vector.tensor_tensor(u2[:], rB[:], rGt[:], Alu.subtract)
            u3 = small.tile([P, 1], f32, tag="u3")
            nc.vector.tensor_scalar(u3[:], u2[:], 4.0, None, Alu.mult)
            u4 = small.tile([P, 1], f32, tag="u4")
            nc.vector.tensor_tensor(u4[:], rA[:], u3[:], Alu.add)
            psS = psp.tile([1, 1], f32, tag="psS")
            nc.tensor.matmul(
                out=psS[:], lhsT=ones[:], rhs=u4[:], start=True, stop=True
            )
            biasc = (4.0 * len(ACT_B_TILES) * FD * P - 4.0 * ROWS * C) / N_TOTAL
            bias_t = small.tile([1, 1], f32, tag="bias")
            nc.vector.memset(bias_t[:], biasc)
            res = small.tile([1, 1], f32, tag="res")
            nc.scalar.activation(
                res[:],
                psS[:],
                mybir.ActivationFunctionType.Identity,
                bias=bias_t[:],
                scale=1.0 / N_TOTAL,
            )
            nc.sync.dma_start(out=out.ap(), in_=res[:])

    nc.compile()
    return nc


_NC_CACHE = None
LAST_RESULTS = None


def kernel(input, target):
    global _NC_CACHE, LAST_RESULTS
    x = np.ascontiguousarray(np.asarray(input, dtype=np.float32))
    tg = np.ascontiguousarray(np.asarray(target).astype(np.int64))
    assert x.shape == (N_TOTAL, C), x.shape
    assert tg.shape == (N_TOTAL,), tg.shape

    if _NC_CACHE is None:
        _NC_CACHE = build_program()
    nc = _NC_CACHE

    # flat element offset of each row's target within its core shard
    offs_all = (
        np.tile(np.arange(ROWS, dtype=np.int64) * C, N_CORES) + tg
    ).astype(np.int32)
    tc_all = tg.astype(np.float32)

    in_maps = [
        {
            "x": x[c * ROWS:(c + 1) * ROWS],
            "og": offs_all[c * ROWS:(c + 1) * ROWS],
            "tc": tc_all[c * ROWS:(c + 1) * ROWS],
        }
        for c in range(N_CORES)
    ]
    res = run_bass_kernel_spmd(nc, in_maps, core_ids=list(range(N_CORES)))
    LAST_RESULTS = res
    total = np.float32(0.0)
    for r in res.results:
        total += np.float32(r["out"].reshape(()))
    return np.asarray(total, dtype=np.float32)


if __name__ == "__main__":
    rng = np.random.default_rng(0)
    xs = rng.standard_normal((N_TOTAL, C), dtype=np.float32)
    ts = rng.integers(0, C, size=(N_TOTAL,)).astype(np.int64)
    got = kernel(xs, ts)
    m = np.where(np.arange(C)[None, :] == ts[:, None], xs, -xs)
    hinge = np.maximum(0.0, 1.0 - m)
    loss = np.where(m >= -1.0, hinge * hinge, -4.0 * m)
    want = loss.sum(dtype=np.float64) / N_TOTAL
    print("got", got, "want", want, "rel", abs(got - want) / abs(want))

